# revision 7
# baseline (speedup 1.0000x reference)
"""TRN2 Bass/Tile kernel for nn_Block_19756849561899 (pre-LN transformer
block: LN -> MHA -> residual -> LN -> MLP(gelu) -> residual).

Self-contained: kernel(**inputs) takes the full fp32 tensors, shards work
across 8 NeuronCores (one batch per core-pair; each core owns half the
sequence as queries and redundantly builds K/V for its batch), compiles a
Bass/Tile program once per process, runs it SPMD, and reassembles the full
output.

Performance structure:
- All dense GEMMs (QKV, V, proj, fc1, fc2) run in fp8e4m3 with DoubleRow
  (0.5 cycles/output-row): weights host-scaled x16 into the e4m3 normal
  range; the x16/x256 products divide out at PSUM evacuation or fold into
  the softmax exp scale.
- softmax exp is split across three engines: Scalar does native exp on
  half the key chunks; DVE and GpSimd approximate exp(y) ~= (a*y+b)^2 on
  the rest (error washes out in the 2048-key softmax normalization).
- LN stats use one-pass bn_stats/bn_aggr on DVE; the rsqrt Newton
  iteration and LN2 application run on GpSimd (no PSUM needed).
- Phase A is a per-4-tile pipeline: x DMA -> stats -> apply -> transpose
  -> V build -> K (and Q) chunk builds, keeping the PE dense from the
  start so the HAM clock gate stays open.
"""

import contextlib

import numpy as np
import ml_dtypes

import concourse.bass as bass
import concourse.mybir as mybir
import concourse.tile as tile
from concourse.masks import make_identity

fp32 = mybir.dt.float32
bf16 = mybir.dt.bfloat16
fp8 = mybir.dt.float8e4
AF = mybir.ActivationFunctionType
ALU = mybir.AluOpType
AX = mybir.AxisListType
DR = mybir.MatmulPerfMode.DoubleRow

C = 384
CS = 3          # C / 128
H = 6
HP = 3          # head pairs
DH = 64
HID = 1536
KS = 12         # HID / 128
VW = 72         # padded V row width (DoubleRow needs 16B-aligned pair stride)
EPS = 1e-6
NBIAS = 24
WS = 16.0       # fp8 weight scale
AS = 16.0       # fp8 attention-output (AT) scale
QS = 256.0      # S psum carries 16q * 16k
PAL = 0.486707  # exp(y) ~= (PAL*y + PBE)^2 on [-1.05, 1.05]
PBE = 1.061244
# per-(h,j) chunk engine map: s=scalar native exp, v=DVE poly, g=GpSimd poly
EXPENG = ["s", "v", "s", "g", "s", "v", "s", "g"]


def build(nc, SEQ=2048, act_fn=AF.Gelu):
    TT = SEQ // 128          # token tiles over full sequence
    QTT = TT // 2            # token tiles in own (query) half
    QLEN = SEQ // 2
    QF = min(512, QLEN)      # q free-dim tile
    NJ = QLEN // QF
    NF = min(512, SEQ)       # seq free-dim tile for K^T build
    NB = QF // 128           # token blocks per q-tile
    CK = 2                   # key tiles per S/exp chunk
    NCH = TT // CK
    chunks = [(k0, min(CK, TT - k0)) for k0 in range(0, TT, CK)]

    xin = nc.dram_tensor("xin", [SEQ, C], fp32, kind="ExternalInput")
    wqk_d = nc.dram_tensor("wqk", [128, CS, 768], fp8, kind="ExternalInput")
    wv_d = nc.dram_tensor("wv", [128, CS, C], fp8, kind="ExternalInput")
    wp_d = nc.dram_tensor("wp", [128, CS, C], fp8, kind="ExternalInput")
    wf1_d = nc.dram_tensor("wf1", [128, CS, HID], fp8, kind="ExternalInput")
    wf2_d = nc.dram_tensor("wf2", [128, KS, C], fp8, kind="ExternalInput")
    bias_d = nc.dram_tensor("bias", [128, NBIAS], fp32, kind="ExternalInput")
    bv_d = nc.dram_tensor("bv", [1, C], fp32, kind="ExternalInput")
    yout = nc.dram_tensor("yout", [QLEN, C], fp32, kind="ExternalOutput")

    xin_t = xin.ap().rearrange("(t p) c -> p t c", p=128)     # [128, TT, C]
    yout_t = yout.ap().rearrange("(t p) c -> p t c", p=128)   # [128, QTT, C]

    with tile.TileContext(nc) as tc, contextlib.ExitStack() as ctx:
        per = ctx.enter_context(tc.tile_pool(name="per", bufs=1))
        drp = ctx.enter_context(tc.tile_pool(name="drp", bufs=2, space="DRAM"))
        xnp = ctx.enter_context(tc.tile_pool(name="xnp", bufs=6))
        xn2p = ctx.enter_context(tc.tile_pool(name="xn2p", bufs=8))
        expp = ctx.enter_context(tc.tile_pool(name="expp", bufs=6))
        plyp = ctx.enter_context(tc.tile_pool(name="plyp", bufs=4))
        rzp = ctx.enter_context(tc.tile_pool(name="rzp", bufs=3))
        ytp = ctx.enter_context(tc.tile_pool(name="ytp", bufs=4))
        hfp = ctx.enter_context(tc.tile_pool(name="hfp", bufs=2))
        sta = ctx.enter_context(tc.tile_pool(name="sta", bufs=1))
        # PSUM: 4 + 2 + 2 banks
        pss = ctx.enter_context(tc.tile_pool(name="pss", bufs=2, space="PSUM"))
        psa = ctx.enter_context(tc.tile_pool(name="psa", bufs=2, space="PSUM"))
        psm = ctx.enter_context(tc.tile_pool(name="psm", bufs=2, space="PSUM"))

        # ---- DMA schedule: wqk first (warmup + QKV), then the first x
        # tiles, then wv/bias/bv, then the rest of x. ----
        wqk = per.tile([128, CS, 768], fp8)
        nc.sync.dma_start(wqk[:], wqk_d.ap())

        x_own = per.tile([128, QTT, C], fp32)
        x_oth = per.tile([128, TT - QTT, C], fp32)

        def xtile(t):
            return x_own[:, t, :] if t < QTT else x_oth[:, t - QTT, :]

        for t in range(4):
            nc.sync.dma_start(xtile(t), xin_t[:, t, :])
        wv = per.tile([128, CS, C], fp8)
        nc.sync.dma_start(wv[:], wv_d.ap())
        bias = per.tile([128, NBIAS], fp32)
        nc.sync.dma_start(bias[:], bias_d.ap())
        bv = per.tile([128, C], fp32)
        nc.sync.dma_start(bv[:], bv_d.ap().to_broadcast([128, C]))
        for t in range(4, TT):
            nc.sync.dma_start(xtile(t), xin_t[:, t, :])

        wp = per.tile([128, CS, C], fp8)
        wf1 = per.tile([128, CS, HID], fp8)
        wf2 = per.tile([128, KS, C], fp8)
        ident = per.tile([128, 128], bf16)
        make_identity(nc, ident)

        # PE warm-up burst so the HAM clock-gate opens before phase A.
        warm = psa.tile([128, NF], fp32, tag="aa", name="warm")
        for _ in range(20):
            nc.tensor.matmul(warm[:, :NF], wqk[:, 0, :128], wqk[:, 0, :NF],
                             start=True, stop=True)
        warmsink = per.tile([128, 1], fp32)
        nc.vector.tensor_copy(warmsink[:, 0:1], warm[:, 0:1])

        x2 = per.tile([128, QTT, C], fp32)
        KT = per.tile([128, HP, SEQ], bf16)
        QT = per.tile([128, HP, QLEN], bf16)
        Vsb = per.tile([128, TT, H, VW], fp8)
        xnT = per.tile([128, CS, SEQ], fp8)
        xn2T = per.tile([128, CS, QLEN], fp8)
        AT = per.tile([128, HP, QLEN], fp8)

        nc.vector.memset(Vsb[:, :, :, DH], 1.0)   # Z ones column

        bv3 = bv.rearrange("p (hp x d) -> p hp x d", x=2, d=DH)
        v3 = Vsb.rearrange("p t (hp x) e -> p t hp x e", x=2)

        # agg cols: 0 mean, 1 var, 2 y(->rstd), 3 tmp, 5 lnb
        bnst = sta.tile([128, TT, 6], fp32)
        agg = sta.tile([128, TT, 6], fp32)

        def ln_stats_tile(xt, t):
            nc.vector.bn_stats(bnst[:, t, :], xt)
            nc.vector.bn_aggr(agg[:, t, 0:2], bnst[:, t, :])

        def ln_group_rstd(sg):
            """rstd via Newton rsqrt on GpSimd: sg [128, G, 6].
            rstd -> col 2, lnb (=-mean*rstd) -> col 5."""
            e = nc.gpsimd
            mean, var = sg[:, :, 0], sg[:, :, 1]
            y, tmp, lnb_ = sg[:, :, 2], sg[:, :, 3], sg[:, :, 5]
            e.tensor_scalar_add(var, var, EPS)
            e.tensor_scalar(y, var, -0.5, 1.5, op0=ALU.mult, op1=ALU.add)
            for _ in range(2):
                e.tensor_tensor(tmp, y, y, ALU.mult)
                e.tensor_tensor(tmp, tmp, var, ALU.mult)
                e.tensor_scalar(tmp, tmp, -0.5, 1.5, op0=ALU.mult, op1=ALU.add)
                e.tensor_tensor(y, y, tmp, ALU.mult)
            # lnb = -mean * rstd (gpsimd has no scalar_tensor_tensor)
            e.tensor_scalar_mul(tmp, mean, -1.0)
            e.tensor_tensor(lnb_, tmp, y, ALU.mult)

        def ln_apply(xt, t, xn_out, eng):
            if eng == "s":
                nc.scalar.activation(
                    xn_out, xt, AF.Identity,
                    bias=agg[:, t, 5:6], scale=agg[:, t, 2:3])
            else:
                nc.gpsimd.tensor_scalar(
                    xn_out, xt, agg[:, t, 2:3], agg[:, t, 5:6],
                    op0=ALU.mult, op1=ALU.add)

        def transpose_to(xn, dstT, t):
            """3 PE transposes of xn [128, C] bf16 into dstT[:, :, t*128...]."""
            ptr = psm.tile([128, max(QF, CS * 128)], bf16, tag="sm", name="ptrA")
            for cs in range(CS):
                nc.tensor.transpose(
                    ptr[:, cs * 128:(cs + 1) * 128],
                    xn[:, cs * 128:(cs + 1) * 128], ident[:])
            src = ptr[:, :CS * 128].rearrange("p (cs n) -> p cs n", n=128)
            nc.vector.tensor_copy(dstT[:, :, t * 128:(t + 1) * 128], src)

        def build_qk_chunk(m, n):
            """one [128 x 512] block of Q^T (m<HP) or K^T: DR + tail matmul."""
            pk = psa.tile([128, NF], fp32, tag="aa", name="pkA")
            nc.tensor.matmul(
                pk[:, :NF], wqk[:, 0:2, m * 128:(m + 1) * 128],
                xnT[:, 0:2, n * NF:(n + 1) * NF],
                start=True, stop=False, perf_mode=DR)
            nc.tensor.matmul(
                pk[:, :NF], wqk[:, 2, m * 128:(m + 1) * 128],
                xnT[:, 2, n * NF:(n + 1) * NF],
                start=False, stop=True)
            if m < HP:
                dst = QT[:, m, n * NF:(n + 1) * NF]
            else:
                dst = KT[:, m - HP, n * NF:(n + 1) * NF]
            nc.scalar.add(dst, pk[:, :NF], bias[:, m:m + 1])

        # ---------------- phase A: LN1 + transposes + V/K/Q builds ----------------
        for c in range(TT // 4):
            for t in range(4 * c, 4 * c + 4):
                ln_stats_tile(xtile(t), t)
            ln_group_rstd(agg[:, 4 * c:4 * c + 4, :])
            for t in range(4 * c, 4 * c + 4):
                xn = xnp.tile([128, C], bf16, tag="xn")
                ln_apply(xtile(t), t, xn[:], "s")
                transpose_to(xn, xnT, t)

                pv = psm.tile([128, max(QF, C)], fp32, tag="sm", name="pvA")
                nc.tensor.matmul(
                    pv[:, :C], xnT[:, 0:2, t * 128:(t + 1) * 128],
                    wv[:, 0:2, :], start=True, stop=False, perf_mode=DR)
                nc.tensor.matmul(
                    pv[:, :C], xnT[:, 2, t * 128:(t + 1) * 128],
                    wv[:, 2, :], start=False, stop=True)
                pv3 = pv[:, :C].rearrange("p (hp x d) -> p hp x d", x=2, d=DH)
                for par in range(2):
                    # Vsb = pv/WS + bv
                    nc.vector.scalar_tensor_tensor(
                        v3[:, t, :, par, :DH], pv3[:, :, par, :], 1.0 / WS,
                        bv3[:, :, par, :], op0=ALU.mult, op1=ALU.add)
            for m in range(HP, 2 * HP):
                build_qk_chunk(m, c)
            if c < NJ:
                for m in range(HP):
                    build_qk_chunk(m, c)

        # deferred weight loads (needed only from proj/MLP onward)
        nc.sync.dma_start(wp[:], wp_d.ap())
        nc.sync.dma_start(wf1[:], wf1_d.ap())
        nc.sync.dma_start(wf2[:], wf2_d.ap())

        # ---------------- attention ----------------
        def pv_pair(po, ech, k0, nk, h):
            # fp8 DoubleRow: one matmul contracts a PAIR of key tiles;
            # lhsT [128, 2, 65], rhs [128, 2, QF] -> out [65, QF]
            if nk == CK:
                nc.tensor.matmul(
                    po[:DH + 1, :], Vsb[:, k0:k0 + 2, h, :DH + 1], ech[:, :2, :],
                    start=(k0 == 0), stop=(k0 + 2 == TT),
                    perf_mode=DR)
            else:
                for i in range(nk):
                    kt = k0 + i
                    nc.tensor.matmul(
                        po[:DH + 1, :], Vsb[:, kt, h, :DH + 1], ech[:, i, :],
                        start=(kt == 0), stop=(kt == TT - 1))

        def exp_chunk(ci, psS, nk):
            ech = expp.tile([128, CK, QF], fp8, tag="ech")
            eng = EXPENG[ci % len(EXPENG)]
            if eng == "s":
                nc.scalar.activation(
                    ech[:, :nk, :], psS[:, :nk * QF], AF.Exp, scale=1.0 / QS)
            else:
                src = psS[:, :nk * QF].rearrange("p (k q) -> p k q", q=QF)
                tp = plyp.tile([128, CK, QF], bf16, tag="tp")
                # t = psS*(PAL/QS) + PBE on DVE (PSUM port), square on the
                # poly engine
                nc.vector.tensor_scalar(
                    tp[:, :nk, :], src, PAL / QS, PBE,
                    op0=ALU.mult, op1=ALU.add)
                e = nc.vector if eng == "v" else nc.gpsimd
                e.tensor_tensor(
                    ech[:, :nk, :], tp[:, :nk, :], tp[:, :nk, :], ALU.mult)
            return ech

        def attention_head(h, j):
            hp, hb = h // 2, (h % 2) * 64
            po = psm.tile([128, QF], fp32, tag="sm", name="po")
            echunks = []
            for ci, (k0, nk) in enumerate(chunks):
                psS = pss.tile([128, CK * QF], fp32, tag="ss")
                for i in range(nk):
                    kt = k0 + i
                    nc.tensor.matmul(
                        psS[:, i * QF:(i + 1) * QF],
                        KT[hb:hb + 64, hp, kt * 128:(kt + 1) * 128],
                        QT[hb:hb + 64, hp, j * QF:(j + 1) * QF],
                        start=True, stop=True)
                echunks.append((exp_chunk(ci, psS, nk), k0, nk))
                if ci > 1:
                    pech, pk0, pnk = echunks[ci - 2]
                    pv_pair(po, pech, pk0, pnk, h)
            for ci in (NCH - 2, NCH - 1):
                pech, pk0, pnk = echunks[ci]
                pv_pair(po, pech, pk0, pnk, h)

            # Z -> DRAM -> broadcast to 64 rows -> fast reciprocal -> mult
            # rz = Z/AS so rzr = AS/Z (bakes the fp8 AT headroom scale in).
            rz = rzp.tile([128, QF], fp32, tag="rz")
            nc.vector.tensor_scalar_mul(rz[64:65, :], po[64:65, :], 1.0 / AS)
            zscr = drp.tile([1, QF], fp32, tag="zscr")
            nc.sync.dma_start(zscr[:], rz[64:65, :])
            rzb = rzp.tile([64, QF], fp32, tag="rzb")
            nc.sync.dma_start(rzb[:], zscr.to_broadcast([64, QF]))
            rzr = rzp.tile([64, QF], fp32, tag="rzr")
            nc.vector.reciprocal_approx_fast(out=rzr[:], in_=rzb[:])
            nc.vector.tensor_tensor(
                AT[hb:hb + 64, hp, j * QF:(j + 1) * QF],
                po[:64, :], rzr[:], ALU.mult)

        def transpose_add(src_sb, dst, res):
            # src_sb [128, NB*128] bf16 -> transpose -> dst = res + src^T
            ptr = psm.tile([128, max(QF, CS * 128)], bf16, tag="sm", name="ptrC")
            for b in range(NB):
                nc.tensor.transpose(
                    ptr[:, b * 128:(b + 1) * 128],
                    src_sb[:, b * 128:(b + 1) * 128], ident[:])
            nc.vector.tensor_tensor(
                dst, ptr[:, :NB * 128].rearrange("p (b n) -> p b n", n=128),
                res, ALU.add)

        def proj_j(j):
            t0 = j * NB
            for m in range(CS):
                pp = psa.tile([128, NF], fp32, tag="aa", name="pp")
                nc.tensor.matmul(
                    pp[:, :QF], wp[:, 0:2, m * 128:(m + 1) * 128],
                    AT[:, 0:2, j * QF:(j + 1) * QF],
                    start=True, stop=False, perf_mode=DR)
                nc.tensor.matmul(
                    pp[:, :QF], wp[:, 2, m * 128:(m + 1) * 128],
                    AT[:, 2, j * QF:(j + 1) * QF],
                    start=False, stop=True)
                y1T = ytp.tile([128, QF], bf16, tag="yT")
                # y1 = pp/(WS*AS) + bias
                nc.vector.tensor_scalar(
                    y1T[:], pp[:, :QF], 1.0 / (WS * AS), bias[:, 6 + m:7 + m],
                    op0=ALU.mult, op1=ALU.add)
                transpose_add(
                    y1T,
                    x2[:, t0:t0 + NB, m * 128:(m + 1) * 128],
                    x_own[:, t0:t0 + NB, m * 128:(m + 1) * 128])

        def ln2_stats_j(j):
            t0 = j * NB
            for t in range(t0, t0 + NB):
                ln_stats_tile(x2[:, t, :], t)
            ln_group_rstd(agg[:, t0:t0 + NB, :])
            tiles = []
            for t in range(t0, t0 + NB):
                xn2 = xn2p.tile([128, C], bf16, tag="xn2")
                ln_apply(x2[:, t, :], t, xn2[:], "g")
                tiles.append(xn2)
            return tiles

        def ln2_tr_j(j, tiles):
            t0 = j * NB
            for i, t in enumerate(range(t0, t0 + NB)):
                transpose_to(tiles[i], xn2T, t)

        def mlp_j(j):
            t0 = j * NB
            hful = hfp.tile([128, KS, QF], fp8, tag="hful")
            for ks in range(KS):
                pf1 = pss.tile([128, CK * QF], fp32, tag="ss", name="pf1")
                nc.tensor.matmul(
                    pf1[:, :QF], wf1[:, 0:2, ks * 128:(ks + 1) * 128],
                    xn2T[:, 0:2, j * QF:(j + 1) * QF],
                    start=True, stop=False, perf_mode=DR)
                nc.tensor.matmul(
                    pf1[:, :QF], wf1[:, 2, ks * 128:(ks + 1) * 128],
                    xn2T[:, 2, j * QF:(j + 1) * QF],
                    start=False, stop=True)
                nc.scalar.activation(
                    hful[:, ks, :], pf1[:, :QF], act_fn,
                    bias=bias[:, 9 + ks:10 + ks], scale=1.0 / WS)
            for m in range(CS):
                pf2 = psa.tile([128, NF], fp32, tag="aa", name="pf2")
                for kp in range(KS // 2):
                    nc.tensor.matmul(
                        pf2[:, :QF],
                        wf2[:, 2 * kp:2 * kp + 2, m * 128:(m + 1) * 128],
                        hful[:, 2 * kp:2 * kp + 2, :],
                        start=(kp == 0), stop=(kp == KS // 2 - 1),
                        perf_mode=DR)
                y2T = ytp.tile([128, QF], bf16, tag="yT")
                nc.vector.tensor_scalar(
                    y2T[:], pf2[:, :QF], 1.0 / (WS * WS), bias[:, 21 + m:22 + m],
                    op0=ALU.mult, op1=ALU.add)
                transpose_add(
                    y2T,
                    x2[:, t0:t0 + NB, m * 128:(m + 1) * 128],
                    x2[:, t0:t0 + NB, m * 128:(m + 1) * 128])
            nc.sync.dma_start(
                yout_t[:, t0:t0 + NB, :], x2[:, t0:t0 + NB, :])

        NJ_ = NJ
        xn2_tiles = {}
        for j in range(NJ_):
            for h in range(H):
                attention_head(h, j)
            proj_j(j)
            xn2_tiles[j] = ln2_stats_j(j)
        # j=0 transposes go right away; later-j transposes are deferred
        # until after mlp_j(j-1) so the PE queue is never head-of-line
        # blocked on LN2 stats of the next j.
        ln2_tr_j(0, xn2_tiles[0])
        for j in range(NJ_):
            mlp_j(j)
            if j + 1 < NJ_:
                ln2_tr_j(j + 1, xn2_tiles[j + 1])
    return nc


def prep_inputs(x, w_qkv, b_qkv, w_proj, b_proj, w_fc1, b_fc1, w_fc2, b_fc2,
                g1, beta1, g2, beta2, n_cores=8):
    """Host-side preprocessing: fold LN affine + attention scale into
    weights/biases, cast to fp8e4m3 (x16 scaling), reshape to SBUF
    layouts, permute x per core."""
    scale_q = DH ** -0.5

    wq = (g1[:, None] * w_qkv[:, :C]) * scale_q
    wk = g1[:, None] * w_qkv[:, C:2 * C]
    wv_ = g1[:, None] * w_qkv[:, 2 * C:]
    bq = (b_qkv[:C] + beta1 @ w_qkv[:, :C]) * scale_q
    bk = b_qkv[C:2 * C] + beta1 @ w_qkv[:, C:2 * C]
    bv_ = b_qkv[2 * C:] + beta1 @ w_qkv[:, 2 * C:]
    wf1_ = g2[:, None] * w_fc1
    bf1_ = b_fc1 + beta2 @ w_fc1

    def kx(w, scale):
        n = w.shape[0] // 128
        return np.ascontiguousarray(
            (w * scale).reshape(n, 128, w.shape[1]).transpose(1, 0, 2)
        ).astype(ml_dtypes.float8_e4m3)

    wqk_l = kx(np.concatenate([wq, wk], axis=1), WS)
    wv_l = kx(wv_, WS)
    wp_l = kx(w_proj, WS)

    wf1_l = kx(wf1_, WS)
    wf2_l = kx(w_fc2, WS * WS)

    bias_h = np.zeros((128, NBIAS), np.float32)
    bias_h[:, 0:3] = (WS * bq).reshape(3, 128).T
    bias_h[:, 3:6] = (WS * bk).reshape(3, 128).T
    bias_h[:, 6:9] = b_proj.reshape(3, 128).T
    bias_h[:, 9:21] = bf1_.reshape(12, 128).T
    bias_h[:, 21:24] = b_fc2.reshape(3, 128).T
    bv_l = np.ascontiguousarray(bv_.reshape(1, C), dtype=np.float32)

    B, N, _ = x.shape
    half = N // 2
    in_maps = []
    for core in range(n_cores):
        b, hf = core // 2, core % 2
        own = x[b, hf * half:(hf + 1) * half]
        other = x[b, (1 - hf) * half:(2 - hf) * half]
        xin_core = np.ascontiguousarray(
            np.concatenate([own, other], axis=0), dtype=np.float32)
        in_maps.append({
            "xin": xin_core, "wqk": wqk_l, "wv": wv_l, "wp": wp_l,
            "wf1": wf1_l, "wf2": wf2_l, "bias": bias_h, "bv": bv_l,
        })
    return in_maps


def assemble_output(results, B, N):
    half = N // 2
    y = np.empty((B, N, C), np.float32)
    for core, r in enumerate(results):
        b, hf = core // 2, core % 2
        y[b, hf * half:(hf + 1) * half] = r["yout"]
    return y


_CACHED = {}


def _get_compiled(SEQ):
    if SEQ not in _CACHED:
        from concourse import bacc
        nc = bacc.Bacc("TRN2", target_bir_lowering=False, debug=False)
        build(nc, SEQ=SEQ)
        nc.compile()
        _CACHED[SEQ] = nc
    return _CACHED[SEQ]


def kernel(x, w_qkv, b_qkv, w_proj, b_proj, w_fc1, b_fc1, w_fc2, b_fc2,
           g1, beta1, g2, beta2):
    from concourse.bass_utils import run_bass_kernel_spmd

    x = np.asarray(x, dtype=np.float32)
    B, N, _ = x.shape
    nc = _get_compiled(N)
    in_maps = prep_inputs(
        x, np.asarray(w_qkv, np.float32), np.asarray(b_qkv, np.float32),
        np.asarray(w_proj, np.float32), np.asarray(b_proj, np.float32),
        np.asarray(w_fc1, np.float32), np.asarray(b_fc1, np.float32),
        np.asarray(w_fc2, np.float32), np.asarray(b_fc2, np.float32),
        np.asarray(g1, np.float32), np.asarray(beta1, np.float32),
        np.asarray(g2, np.float32), np.asarray(beta2, np.float32),
        n_cores=2 * B)
    res = run_bass_kernel_spmd(
        nc, in_maps, core_ids=list(range(2 * B)), trace=False)
    return assemble_output(res.results, B=B, N=N)
